# revision 28
# baseline (speedup 1.0000x reference)
"""Trainium2 Bass kernel for ContextAM (sigmoid spatial attention + CBAM channel gate).

Reference computation (per batch b):
  Q = wq @ X + bq   [8, N]      (X = x[b] as [64, N], N = 96*96 = 9216)
  K = wk @ X + bk   [8, N]
  V = wv @ X + bv   [64, N]
  att = sigmoid(Q^T K)          [N, N]   -- never materialized to HBM
  out = V @ att^T + X           [64, N]
  scale = sigmoid(mlp(mean_n(out)) + mlp(max_n(out)))   [64]
  y = out * scale[:, None]

Sharding: 8 cores = (batch b = core//2) x (n-half h = core%2). Each core
computes out[:, h*4608:(h+1)*4608] for its batch.

The kernel is SIGMOID-bound: 4608*9216 = 42.5M att elements per core must
pass through the scalar (ACT) engine at 1 elem/cycle/lane (1.2 GHz), a hard
~300us floor. The design keeps a gapless sigmoid stream:
  - m-tiles processed in TRIADS (3x128 rows): 3 row-packed energy matmuls
    (K=8 at PE row groups 0/32/64) fill one [128,1536] PSUM tile (3 banks),
    consumed by ONE sigmoid instruction (FD=1536 amortizes the PSUM access
    latency). Two triad buffers double-buffer (6 banks).
  - out matmuls (K=128, M=64) col-packed at (0,0)/(0,64) accumulate into a
    1-bank [128,512] accumulator per 512-col n-chunk; 2 accumulator slots
    (2 banks) let chunk finalization overlap the next chunk.
  - PE work (~0.7us/triad) hides under each 1.5us sigmoid; emission is
    software-pipelined (energy of triad k ahead of out-matmuls of k-LAG)
    so the PE queue never blocks the energy feed.
CBAM stats are accumulated per-chunk on the DVE and exchanged between the
two cores of a batch pair with a tiny pairwise AllGather at the end.
"""

import numpy as np

import concourse.bacc as bacc
import concourse.mybir as mybir
import concourse.tile as tile
from concourse.bass_utils import run_bass_kernel_spmd

F32 = mybir.dt.float32
BF16 = mybir.dt.bfloat16

B, C, H, W = 4, 64, 96, 96
N = H * W            # 9216
C8 = C // 8          # 8
R = C // 16          # 4
N_CORES = 8
NHALF = N // 2       # 4608 columns of out per core
MT = 128             # m-tile rows
NT = N // MT         # 72 m-tiles
TRI = 3              # m-tiles per sigmoid triad
NTRI = NT // TRI     # 24 triads per n-chunk
CH = 512             # n-chunk columns (one PSUM bank)
NCH = NHALF // CH    # 9 n-chunks
LAG = 9              # sigmoid/out stream lags energy emission by LAG triads
NPV = NT // 8        # 9 groups of 8 V^T tiles


def build_nc():
    nc = bacc.Bacc("TRN2", target_bir_lowering=False, debug=False,
                   enable_asserts=True, num_devices=N_CORES)

    xbb = nc.dram_tensor("xbb", [C + 1, N], BF16, kind="ExternalInput").ap()
    wqTb = nc.dram_tensor("wqTb", [C + 1, 32], BF16, kind="ExternalInput").ap()
    wkTb = nc.dram_tensor("wkTb", [C + 1, 32], BF16, kind="ExternalInput").ap()
    wvTb = nc.dram_tensor("wvTb", [C + 1, C], BF16, kind="ExternalInput").ap()
    w1T = nc.dram_tensor("w1T", [C, R], F32, kind="ExternalInput").ap()
    w2T = nc.dram_tensor("w2T", [R, C], F32, kind="ExternalInput").ap()

    y = nc.dram_tensor("y", [C, NHALF], F32, kind="ExternalOutput").ap()

    cc_in = nc.dram_tensor("cc_in", [1, 2 * C], F32).ap()
    cc_out = nc.dram_tensor("cc_out", [2, 2 * C], F32).ap()
    cc_win = nc.dram_tensor("cc_win", [1, 2], F32).ap()
    cc_wout = nc.dram_tensor("cc_wout", [2, 2], F32).ap()

    with tile.TileContext(nc) as tc:
        with (
            tc.tile_pool(name="const", bufs=1) as cpool,
            tc.tile_pool(name="att", bufs=LAG + 2) as apool,
            tc.tile_pool(name="pe", bufs=2, space="PSUM") as epool,
            tc.tile_pool(name="po", bufs=2, space="PSUM") as opool,
        ):
            # ---- resident SBUF tensors -------------------------------------
            # X columns are ROTATED host-side so this core's own n-half sits
            # at columns 0:NHALF (m is only ever summed over, so any
            # consistent column permutation of K/V is fine).
            X = cpool.tile([C + 1, N], BF16)       # x[b] plus ones row
            Kt = cpool.tile([72, N], BF16)         # K strips at partitions 0-7/32-39/64-71
            Qt = cpool.tile([72, NHALF], BF16)     # Q strips likewise
            VT = cpool.tile([MT, NT * C], BF16)    # V^T as 72 tiles of [128, 64]
            OUT = cpool.tile([C, NHALF], F32)      # attention out + x
            stat_s = cpool.tile([C, NCH], F32)     # per-chunk row sums
            stat_m = cpool.tile([C, NCH], F32)     # per-chunk row maxes

            wq_s = cpool.tile([C + 1, 32], BF16)   # 8 real cols + 24 zero
            wk_s = cpool.tile([C + 1, 32], BF16)
            wv_s = cpool.tile([C + 1, C], BF16)
            w1_s = cpool.tile([C, R], F32)
            w2_s = cpool.tile([R, C], F32)

            # Input DMAs: each hardware DMA ring is FIFO at ~35-45 GB/s, so
            # the 1.2MB x tensor is split across the three available rings
            # (SP / ACT / gpsimd) in need-order, small head piece first.
            nc.sync.dma_start(w1_s[:], w1T[:])
            nc.sync.dma_start(wq_s[:], wqTb[:])
            nc.sync.dma_start(wk_s[:], wkTb[:])
            XP = 2304
            nc.sync.dma_start(X[:, 0:1024], xbb[:, 0:1024])
            nc.sync.dma_start(X[:, 1024:XP], xbb[:, 1024:XP])
            nc.sync.dma_start(wv_s[:], wvTb[:])
            nc.sync.dma_start(w2_s[:], w2T[:])
            nc.scalar.dma_start(X[:, XP:2 * XP], xbb[:, XP:2 * XP])
            nc.gpsimd.dma_start(X[:, NHALF:NHALF + XP], xbb[:, NHALF:NHALF + XP])
            nc.gpsimd.dma_start(X[:, NHALF + XP:N], xbb[:, NHALF + XP:N])

            # Preload the sigmoid ACT table while DMAs run, and warm the
            # collective stream so the real AllGather launches fast.
            warm_o = cpool.tile([1, 2], F32)
            nc.scalar.activation(warm_o[:], w1_s[0:1, 0:2],
                                 mybir.ActivationFunctionType.Sigmoid)
            nc.sync.dma_start(cc_win[0:1, :], w1_s[0:1, 0:2])
            nc.gpsimd.collective_compute(
                "AllGather", mybir.AluOpType.bypass,
                ins=[cc_win.opt()], outs=[cc_wout.opt()],
                replica_groups=[[0, 1], [2, 3], [4, 5], [6, 7]])

            # ---- Q/K projections, col-packed into all 3 partition strips ---
            # wq/wk are zero-padded to 32 cols; three concurrent matmuls at
            # col-groups (0,0)/(0,32)/(0,64) write Q (K) directly at
            # partitions 0-7 / 32-39 / 64-71, so one [72,CH] cast per chunk
            # replaces per-strip DMAs. Q casts ride DVE, head K casts ride
            # ACT (idle before the sigmoid stream); K chunks 9-17 are woven
            # into the main loop with DVE casts.
            def emit_proj(dst, w_s, j, cast_engine, nm):
                pp = epool.tile([96, CH], F32, tag="pe", name=f"pp{nm}{j}")
                for i in range(3):
                    nc.tensor.matmul(pp[32 * i:32 * i + 32, :], w_s[:],
                                     X[:, j * CH:(j + 1) * CH],
                                     start=True, stop=True,
                                     tile_position=(0, 32 * i))
                if cast_engine == "act":
                    nc.scalar.copy(dst[0:72, j * CH:(j + 1) * CH], pp[0:72, :])
                else:
                    nc.vector.tensor_copy(dst[0:72, j * CH:(j + 1) * CH],
                                          pp[0:72, :])

            # Head: only what E_0..E_3 need (Q chunk 0, K chunks 0-2); all
            # other projections and the V^T groups are woven into the main
            # loop at stages that respect each tile's read deadline
            # (K chunk c first read by triad ceil((4c-2)/3); pv group g by
            # out-stage ceil(8g/3)+LAG; late Q chunk j by stage 24j).
            emit_proj(Qt, wq_s, 0, "dve", "q")
            for j in range(3):
                emit_proj(Kt, wk_s, j, "dve", "k")

            klate = {k: 3 + k for k in range(15)}          # K chunks 3-17
            pvmap = {3: 0, 6: 1, 9: 2, 12: 3, 15: 4, 16: 5,
                     17: 6, 18: 7, 19: 8}
            qlate = {20 + j: j for j in range(1, NCH)}     # Q chunks 1-8

            # ---- main flash loop -------------------------------------------
            # Flattened over 9 n-chunks x 24 triads = 216 stages.
            # emit_energy(k) runs LAG stages ahead of emit_sig_out(k) so the
            # PE queue (strict FIFO) always has the next energy group ready
            # before the out-matmuls that wait on sigmoid results.
            TOTAL = NCH * NTRI
            pe_tiles = {}
            po_cur = {}

            def emit_energy(k):
                jc, q = divmod(k, NTRI)
                c0 = jc * CH
                pe = epool.tile([MT, TRI * CH], F32, tag="pe", name=f"pe{k}")
                for i in range(TRI):
                    t = TRI * q + i
                    nc.tensor.matmul(
                        pe[:, i * CH:(i + 1) * CH],
                        Kt[32 * i:32 * i + C8, t * MT:(t + 1) * MT],
                        Qt[32 * i:32 * i + C8, c0:c0 + CH],
                        start=True, stop=True, tile_position=(32 * i, 0))
                pe_tiles[k] = pe

            def emit_pv(g):
                # V^T tiles 8g..8g+7 batched into one PSUM tile so a
                # single DVE copy evacuates 8 tiles.
                pvb = epool.tile([MT, CH], F32, tag="pe", name=f"pvb{g}")
                for i in range(8):
                    t = 8 * g + i
                    nc.tensor.matmul(pvb[:, i * C:(i + 1) * C],
                                     X[:, t * MT:(t + 1) * MT], wv_s[:],
                                     start=True, stop=True)
                nc.vector.tensor_copy(VT[:, g * CH:(g + 1) * CH], pvb[:])

            def emit_sig_out(k):
                jc, q = divmod(k, NTRI)
                c0 = jc * CH
                pe = pe_tiles.pop(k)
                at = apool.tile([MT, TRI * CH], BF16, tag="att")
                nc.scalar.activation(at[:], pe[:],
                                     mybir.ActivationFunctionType.Sigmoid)
                if q == 0:
                    po_cur[0] = opool.tile([MT, CH], F32, tag="po",
                                           name=f"po{jc}")
                po = po_cur[0]
                t0 = TRI * q
                nc.tensor.matmul(po[0:C, :], VT[:, t0 * C:(t0 + 1) * C],
                                 at[:, 0:CH],
                                 start=(q == 0), stop=False,
                                 tile_position=(0, 0))
                nc.tensor.matmul(po[C:MT, :], VT[:, (t0 + 1) * C:(t0 + 2) * C],
                                 at[:, CH:2 * CH],
                                 start=(q == 0), stop=(q == NTRI - 1),
                                 tile_position=(0, 64))
                nc.tensor.matmul(po[0:C, :], VT[:, (t0 + 2) * C:(t0 + 3) * C],
                                 at[:, 2 * CH:3 * CH],
                                 start=False, stop=(q == NTRI - 1),
                                 tile_position=(0, 0))
                if q == NTRI - 1:
                    sl = slice(c0, c0 + CH)
                    nc.vector.tensor_add(OUT[:, sl], po[0:C, :], X[0:C, sl])
                    nc.vector.tensor_add(OUT[:, sl], OUT[:, sl], po[C:MT, :])
                    nc.vector.reduce_sum(stat_s[:, jc:jc + 1], OUT[:, sl],
                                         axis=mybir.AxisListType.X)
                    nc.vector.reduce_max(stat_m[:, jc:jc + 1], OUT[:, sl],
                                         axis=mybir.AxisListType.X)

            for k in range(TOTAL):
                emit_energy(k)
                if k in klate:
                    emit_proj(Kt, wk_s, klate[k], "dve", "kl")
                if k in qlate:
                    emit_proj(Qt, wq_s, qlate[k], "dve", "ql")
                if k in pvmap:
                    emit_pv(pvmap[k])
                if k >= LAG:
                    emit_sig_out(k - LAG)
            for k in range(TOTAL - LAG, TOTAL):
                emit_sig_out(k)

            # ---- CBAM channel gate -----------------------------------------
            st = cpool.tile([C, 2], F32)
            nc.vector.reduce_sum(st[:, 0:1], stat_s[:], axis=mybir.AxisListType.X)
            nc.vector.reduce_max(st[:, 1:2], stat_m[:], axis=mybir.AxisListType.X)
            nc.sync.dma_start(cc_in[0:1, 0:C], st[:, 0:1])
            nc.sync.dma_start(cc_in[0:1, C:2 * C], st[:, 1:2])
            nc.gpsimd.collective_compute(
                "AllGather", mybir.AluOpType.bypass,
                ins=[cc_in.opt()], outs=[cc_out.opt()],
                replica_groups=[[0, 1], [2, 3], [4, 5], [6, 7]])

            sums2 = cpool.tile([C, 2], F32)
            maxs2 = cpool.tile([C, 2], F32)
            nc.sync.dma_start(sums2[:, 0:1], cc_out[0:1, 0:C])
            nc.sync.dma_start(sums2[:, 1:2], cc_out[1:2, 0:C])
            nc.sync.dma_start(maxs2[:, 0:1], cc_out[0:1, C:2 * C])
            nc.sync.dma_start(maxs2[:, 1:2], cc_out[1:2, C:2 * C])

            avgmx = cpool.tile([C, 2], F32)
            nc.vector.reduce_sum(avgmx[:, 0:1], sums2[:], axis=mybir.AxisListType.X)
            nc.vector.tensor_scalar_mul(avgmx[:, 0:1], avgmx[:, 0:1], 1.0 / N)
            nc.vector.reduce_max(avgmx[:, 1:2], maxs2[:], axis=mybir.AxisListType.X)

            ph = epool.tile([R, 2], F32, tag="pe")
            nc.tensor.matmul(ph[:], w1_s[:], avgmx[:], start=True, stop=True)
            hrelu = cpool.tile([R, 2], F32)
            nc.vector.tensor_scalar_max(hrelu[:], ph[:], 0.0)
            ps = epool.tile([C, 2], F32, tag="pe")
            nc.tensor.matmul(ps[:], w2_s[:], hrelu[:], start=True, stop=True)
            ssum = cpool.tile([C, 1], F32)
            nc.vector.reduce_sum(ssum[:], ps[:], axis=mybir.AxisListType.X)
            scale = cpool.tile([C, 1], F32)
            nc.scalar.activation(scale[:], ssum[:],
                                 mybir.ActivationFunctionType.Sigmoid)

            # scale + store, one piece per DMA ring
            PC = NHALF // 3
            dmas = [nc.sync.dma_start, nc.scalar.dma_start, nc.gpsimd.dma_start]
            for p in range(3):
                sl = slice(p * PC, (p + 1) * PC)
                nc.vector.tensor_scalar_mul(OUT[:, sl], OUT[:, sl], scale[:])
                dmas[p](y[:, sl], OUT[:, sl])

    nc.compile()
    return nc


_NC_CACHE = None


def _get_nc():
    global _NC_CACHE
    if _NC_CACHE is None:
        _NC_CACHE = build_nc()
    return _NC_CACHE


def build_in_maps(inputs):
    import ml_dtypes
    bf16 = ml_dtypes.bfloat16

    x = np.ascontiguousarray(np.asarray(inputs["x"], np.float32))
    wq = np.asarray(inputs["wq"], np.float32)
    bq = np.asarray(inputs["bq"], np.float32)
    wk = np.asarray(inputs["wk"], np.float32)
    bk = np.asarray(inputs["bk"], np.float32)
    wv = np.asarray(inputs["wv"], np.float32)
    bv = np.asarray(inputs["bv"], np.float32)
    ca_w1 = np.asarray(inputs["ca_w1"], np.float32)
    ca_w2 = np.asarray(inputs["ca_w2"], np.float32)

    # zero-pad the 8 Q/K output channels to 32 so col-packed projection
    # matmuls cover whole 32-partition groups
    def pad32(w, b):
        wb = np.concatenate([w.T, b[None, :]], axis=0)        # [65, 8]
        out = np.zeros((C + 1, 32), np.float32)
        out[:, :C8] = wb
        return np.ascontiguousarray(out.astype(bf16))

    wqTb = pad32(wq, bq)
    wkTb = pad32(wk, bk)
    wvTb = np.ascontiguousarray(
        np.concatenate([wv.T, bv[None, :]], axis=0).astype(bf16))
    w1T = np.ascontiguousarray(ca_w1.T)
    w2T = np.ascontiguousarray(ca_w2.T)

    xf = x.reshape(B, C, N)
    ones = np.ones((1, N), np.float32)
    in_maps = []
    for core in range(N_CORES):
        b, h = core // 2, core % 2
        xb1 = np.concatenate([xf[b], ones], axis=0)     # [65, N]
        # rotate columns so this core's own n-half is at cols 0:NHALF
        if h == 1:
            xb1 = np.concatenate([xb1[:, NHALF:], xb1[:, :NHALF]], axis=1)
        in_maps.append({
            "xbb": np.ascontiguousarray(xb1.astype(bf16)),
            "wqTb": wqTb, "wkTb": wkTb, "wvTb": wvTb,
            "w1T": w1T, "w2T": w2T,
        })
    return in_maps


def assemble_output(results):
    out = np.empty((B, C, N), np.float32)
    for core in range(N_CORES):
        b, h = core // 2, core % 2
        out[b][:, h * NHALF:(h + 1) * NHALF] = results[core]["y"]
    return out.reshape(B, C, H, W)


def kernel(**inputs):
    nc = _get_nc()
    res = run_bass_kernel_spmd(nc, build_in_maps(inputs), list(range(N_CORES)))
    return assemble_output(res.results)
